# revision 12
# baseline (speedup 1.0000x reference)
"""MoE-routed 3x3 conv (MixedLayerWithArc) on 8 TRN2 NeuronCores.

Reference semantics: out[i] = conv3x3(x[i], W[sample_arc[i]], b[sample_arc[i]]).

Hybrid Winograd/direct kernel, DMA-roofline-driven.  The kernel is
DMA-bound (~360 GB/s/core pipe), so the split minimizes per-core HBM bytes:

  * 6 samples/core via Winograd F(4x4,3x3), all transforms on the host:
    device does 36 per-position fp16 GEMMs against ONE branch's transformed
    weights (4.72 MB).  Input inflates 2.25x but PE columns drop 4x.
  * 2 samples/core via DIRECT conv (9 shifted-window taps): raw fp16
    weights are 4x smaller (1.18 MB/sample-branch), raw x/y are 1x.
    Costs 4x PE columns for those samples, but the PE has slack.

  Per-core bytes: wino-x 7.1 + direct-x 1.2 + W (4.7+2.4) + out (8.1)
  = ~23.4 MB vs 28.3 MB for all-Winograd (5,3).  PE ~71 us, pipe ~65 us.

  Streaming structure (lessons from traces):
  * in-DMAs on SP/ACT queues in pos chunks (small first chunk -> early PE
    start); direct-sample data injected mid-stream (PE needs it last).
  * evictions alternate DVE/ACT (gpsimd can't read PSUM).
  * out-DMAs ONLY on the gpsimd (SWDGE) queue: in-order DMA queues would
    serialize them behind the in-stream and stall PSUM->PE mid-kernel.
  * On-device output transforms don't pay: DVE scalar_tensor_tensor is
    ~1.5us/op and Pool tensor ops are Q7 software (~1.2us/op).

Falls back to the all-Winograd uniform-slot program for routings where no
(6,2) packing exists.
"""
import os

import numpy as np

B, C, H, W_ = 64, 256, 32, 32
NB = 4                     # branches
NCORES = 8
SPC = B // NCORES          # samples per core
P = 128                    # partition tile
CT = C // P                # channel tiles (2)
NT = H // 4                # 8 winograd tiles per axis
NTILES = NT * NT           # 64 tiles per sample
NPOS = 36                  # 6x6 winograd positions
WARMUP = 16
OCH = 6                    # out-DMA grouping (pos per block)
IN_CHUNKS = [(0, 2), (2, 6), (6, 12), (12, 18), (18, 24), (24, 30), (30, 36)]

NW = 6                     # wino samples per core (hybrid)
ND = 2                     # direct samples per core (hybrid)
WCOLS = NW * NTILES        # 384 wino matmul columns
HP, WP = H + 2, W_ + 2     # padded spatial for the direct path

BT = np.array([
    [4, 0, -5, 0, 1, 0],
    [0, -4, -4, 1, 1, 0],
    [0, 4, -4, -1, 1, 0],
    [0, -2, -1, 2, 1, 0],
    [0, 2, -1, -2, 1, 0],
    [0, 4, 0, -5, 0, 1]], dtype=np.float32)
G = np.array([
    [1 / 4, 0, 0],
    [-1 / 6, -1 / 6, -1 / 6],
    [-1 / 6, 1 / 6, -1 / 6],
    [1 / 24, 1 / 12, 1 / 6],
    [1 / 24, -1 / 12, 1 / 6],
    [0, 0, 1]], dtype=np.float32)
AT = np.array([
    [1, 1, 1, 1, 1, 0],
    [0, 1, -1, 2, -2, 0],
    [0, 1, 1, 4, 4, 0],
    [0, 1, -1, 8, -8, 1]], dtype=np.float32)

PATTERNS = [
    (8,), (5, 3), (6, 2), (7, 1), (4, 4),
    (4, 3, 1), (3, 3, 2), (4, 2, 2), (5, 2, 1), (6, 1, 1),
    (2, 2, 2, 2), (3, 2, 2, 1), (3, 3, 1, 1), (4, 2, 1, 1), (5, 1, 1, 1),
    (2, 2, 2, 1, 1), (4, 1, 1, 1, 1), (2, 2, 1, 1, 1, 1),
    (2, 1, 1, 1, 1, 1, 1), (1,) * 8,
]

TRACE = False
TRACE_DIR = None
LAST_RESULTS = None

_prog_cache = {}


# --------------------------- routing ---------------------------------------

def _plan_hybrid(arc):
    """Per core: NW wino samples of one branch + ND direct samples (any).
    Returns (wino_branch [NCORES], perm [B]) or None if infeasible."""
    counts = np.bincount(arc, minlength=NB).tolist()

    def dfs(b, left, cs):
        if b == NB:
            return cs if left == 0 else None
        for c in range(min(left, counts[b] // NW), -1, -1):
            r = dfs(b + 1, left - c, cs + [c])
            if r is not None:
                return r
        return None

    cs = dfs(0, NCORES, [])
    if cs is None:
        return None
    wino_branch = np.array(sum(([br] * cs[br] for br in range(NB)), []),
                           np.int64)
    pools = [list(np.nonzero(arc == br)[0]) for br in range(NB)]
    perm = np.empty(B, np.int64)
    for c in range(NCORES):
        br = wino_branch[c]
        for k in range(NW):
            perm[SPC * c + k] = pools[br].pop()
    rest = [i for br in range(NB) for i in pools[br]]
    for c in range(NCORES):
        for k in range(ND):
            perm[SPC * c + NW + k] = rest[ND * c + k]
    return wino_branch, perm


def _pattern_assign(counts, sizes):
    K = len(sizes)
    caps = [NCORES] * K
    m = []

    def branch_vecs(j, rem):
        if j == K:
            if rem == 0:
                yield []
            return
        for v in range(min(caps[j], rem // sizes[j]), -1, -1):
            for rest in branch_vecs(j + 1, rem - v * sizes[j]):
                yield [v] + rest

    def dfs(b):
        if b == NB:
            return all(c == 0 for c in caps)
        for v in branch_vecs(0, counts[b]):
            for j in range(K):
                caps[j] -= v[j]
            m.append(v)
            if dfs(b + 1):
                return True
            m.pop()
            for j in range(K):
                caps[j] += v[j]
        return False

    return m if dfs(0) else None


def _plan_wino(arc):
    counts = np.bincount(arc, minlength=NB).tolist()
    for sizes in PATTERNS:
        m = _pattern_assign(counts, sizes)
        if m is not None:
            break
    assert m is not None
    K = len(sizes)
    slot_branches = np.empty((NCORES, K), np.int64)
    for j in range(K):
        lst = []
        for br in range(NB):
            lst += [br] * m[br][j]
        slot_branches[:, j] = lst
    pools = [list(np.nonzero(arc == br)[0]) for br in range(NB)]
    perm = np.empty(B, np.int64)
    i = 0
    for c in range(NCORES):
        for j, sz in enumerate(sizes):
            br = slot_branches[c, j]
            for _ in range(sz):
                perm[i] = pools[br].pop()
                i += 1
    return sizes, slot_branches, perm


# --------------------------- host transforms -------------------------------

def _wino_input(x_sel):
    """[n,C,H,W] -> transformed V [n,CT,P,36, n?..]: returns
    [CT, P, 36, n*64] f16 stacked per core by the caller."""
    n = x_sel.shape[0]
    xp = np.zeros((n, C, H + 2, W_ + 2), np.float32)
    xp[:, :, 1:1 + H, 1:1 + W_] = x_sel
    win = np.lib.stride_tricks.sliding_window_view(
        xp, (6, 6), axis=(2, 3))[:, :, ::4, ::4]      # [n,C,8,8,6,6]
    V = np.matmul(np.matmul(BT, win), BT.T)           # [n,C,8,8,xi,nu]
    V = V.reshape(n, CT, P, NT, NT, 6, 6)
    # -> [CT, P, pos, n, ty, tx]
    return V.transpose(1, 2, 5, 6, 0, 3, 4).reshape(CT, P, NPOS, n * NTILES)


# --------------------------- device programs -------------------------------

def _build_hybrid():
    import concourse.tile as tile
    from concourse import bacc, mybir

    nc = bacc.Bacc("TRN2", target_bir_lowering=False, debug=False,
                   num_devices=NCORES)
    f32 = mybir.dt.float32
    f16 = mybir.dt.float16

    xw_d = nc.dram_tensor("xw", [CT, P, NPOS, WCOLS], f16,
                          kind="ExternalInput").ap()
    ww_d = nc.dram_tensor("ww", [CT, P, NPOS, CT, P], f16,
                          kind="ExternalInput").ap()
    xd_d = nc.dram_tensor("xd", [ND, CT, P, HP, WP], f16,
                          kind="ExternalInput").ap()
    wd_d = nc.dram_tensor("wd", [ND, CT, P, 9, CT, P], f16,
                          kind="ExternalInput").ap()
    ow_d = nc.dram_tensor("ow", [CT, NPOS // OCH, P, OCH, WCOLS], f16,
                          kind="ExternalOutput").ap()
    od_d = nc.dram_tensor("od", [ND, CT, P, H * W_], f16,
                          kind="ExternalOutput").ap()

    with tile.TileContext(nc) as tc:
        with tc.tile_pool(name="xpool", bufs=1) as xpool, \
             tc.tile_pool(name="wpool", bufs=1) as wpool, \
             tc.tile_pool(name="opool", bufs=4) as opool, \
             tc.tile_pool(name="psum", bufs=8, space="PSUM") as psum_pool:

            scratch = xpool.tile([P, P], f16, name="scratch", tag="scr")
            nc.gpsimd.memset(scratch[:], 0.0)
            ps_warm = psum_pool.tile([P, 512], f32, name="ps_warm", tag="ps")
            for _ in range(WARMUP):
                nc.tensor.matmul(ps_warm[:, :P], scratch[:], scratch[:],
                                 start=True, stop=True, skip_group_check=True)

            xws = [xpool.tile([P, NPOS, WCOLS], f16, name=f"xw{ci}",
                              tag=f"xw{ci}") for ci in range(CT)]
            wws = [wpool.tile([P, NPOS, CT, P], f16, name=f"ww{ci}",
                              tag=f"ww{ci}") for ci in range(CT)]
            xds = [[xpool.tile([P, HP, WP], f16, name=f"xd{d}_{ci}",
                               tag=f"xd{d}_{ci}") for ci in range(CT)]
                   for d in range(ND)]
            wds = [[wpool.tile([P, 9, CT, P], f16, name=f"wd{d}_{ci}",
                               tag=f"wd{d}_{ci}") for ci in range(CT)]
                   for d in range(ND)]

            for k, (lo, hi) in enumerate(IN_CHUNKS):
                sl = slice(lo, hi)
                for ci in range(CT):
                    nc.sync.dma_start(xws[ci][:, sl], xw_d[ci][:, sl])
                    nc.scalar.dma_start(wws[ci][:, sl], ww_d[ci][:, sl])
                if k == 3:
                    # direct-path data lands mid-stream; PE needs it last
                    for d in range(ND):
                        for ci in range(CT):
                            nc.sync.dma_start(xds[d][ci][:], xd_d[d, ci])
                            nc.scalar.dma_start(wds[d][ci][:], wd_d[d, ci])

            # ---- winograd part: 72 banks of N=384 ----
            ots = {}
            for pos in range(NPOS):
                for co_t in range(CT):
                    ps = psum_pool.tile([P, 512], f32,
                                        name=f"ps{pos}_{co_t}", tag="ps")
                    for ci in range(CT):
                        nc.tensor.matmul(
                            ps[:, :WCOLS],
                            lhsT=wws[ci][:, pos, co_t, :],
                            rhs=xws[ci][:, pos, :],
                            start=(ci == 0), stop=(ci == CT - 1))
                    if pos % OCH == 0:
                        ots[co_t] = opool.tile([P, OCH, WCOLS], f16,
                                               name=f"ot{pos}_{co_t}",
                                               tag="ot")
                    ot = ots[co_t]
                    if (pos * CT + co_t) % 2 == 0:
                        nc.vector.tensor_copy(ot[:, pos % OCH, :],
                                              ps[:, :WCOLS])
                    else:
                        nc.scalar.copy(ot[:, pos % OCH, :], ps[:, :WCOLS])
                    if pos % OCH == OCH - 1:
                        nc.gpsimd.dma_start(ow_d[co_t, pos // OCH], ot[:])

            # ---- direct part: 2 samples x 2 co_t x 2 row-chunks ----
            for d in range(ND):
                for co_t in range(CT):
                    oy = opool.tile([P, H * W_], f16, name=f"oy{d}_{co_t}",
                                    tag="oy")
                    for ch in range(2):
                        ps = psum_pool.tile([P, 512], f32,
                                            name=f"pd{d}_{co_t}_{ch}",
                                            tag="ps")
                        for ci in range(CT):
                            for tap in range(9):
                                dy, dx = divmod(tap, 3)
                                rhs = xds[d][ci][:, 16 * ch + dy:
                                                 16 * ch + dy + 16,
                                                 dx: dx + W_]
                                nc.tensor.matmul(
                                    ps[:],
                                    lhsT=wds[d][ci][:, tap, co_t, :],
                                    rhs=rhs,
                                    start=(ci == 0 and tap == 0),
                                    stop=(ci == CT - 1 and tap == 8))
                        if (d * CT + co_t + ch) % 2 == 0:
                            nc.vector.tensor_copy(
                                oy[:, 512 * ch: 512 * (ch + 1)], ps[:])
                        else:
                            nc.scalar.copy(
                                oy[:, 512 * ch: 512 * (ch + 1)], ps[:])
                    nc.gpsimd.dma_start(od_d[d, co_t], oy[:])
    nc.compile()
    return nc


def _build_wino(sizes):
    import concourse.tile as tile
    from concourse import bacc, mybir

    K = len(sizes)
    NCOLS = 512
    bounds = [0]
    for s in sizes:
        bounds.append(bounds[-1] + s * NTILES)

    nc = bacc.Bacc("TRN2", target_bir_lowering=False, debug=False,
                   num_devices=NCORES)
    f32 = mybir.dt.float32
    f16 = mybir.dt.float16

    xt_d = nc.dram_tensor("xt", [CT, P, NPOS, NCOLS], f16,
                          kind="ExternalInput").ap()
    wt_d = nc.dram_tensor("wt", [K, CT, P, NPOS, CT, P], f16,
                          kind="ExternalInput").ap()
    out_d = nc.dram_tensor("out", [CT, NPOS // OCH, P, OCH, NCOLS], f16,
                           kind="ExternalOutput").ap()

    with tile.TileContext(nc) as tc:
        with tc.tile_pool(name="xpool", bufs=1) as xpool, \
             tc.tile_pool(name="wpool", bufs=1) as wpool, \
             tc.tile_pool(name="opool", bufs=4) as opool, \
             tc.tile_pool(name="psum", bufs=8, space="PSUM") as psum_pool:

            scratch = xpool.tile([P, P], f16, name="scratch", tag="scr")
            nc.gpsimd.memset(scratch[:], 0.0)
            ps_warm = psum_pool.tile([P, NCOLS], f32, name="ps_warm", tag="ps")
            for _ in range(WARMUP):
                nc.tensor.matmul(ps_warm[:, :P], scratch[:], scratch[:],
                                 start=True, stop=True, skip_group_check=True)

            xts = [xpool.tile([P, NPOS, NCOLS], f16, name=f"xt{ci}",
                              tag=f"xt{ci}") for ci in range(CT)]
            wts = [[wpool.tile([P, NPOS, CT, P], f16, name=f"wt{j}_{ci}",
                               tag=f"wt{j}_{ci}")
                    for ci in range(CT)] for j in range(K)]

            for lo, hi in IN_CHUNKS:
                sl = slice(lo, hi)
                for ci in range(CT):
                    nc.sync.dma_start(xts[ci][:, sl], xt_d[ci][:, sl])
                for j in range(K):
                    for ci in range(CT):
                        nc.scalar.dma_start(wts[j][ci][:, sl],
                                            wt_d[j, ci][:, sl])

            ots = {}
            for pos in range(NPOS):
                for co_t in range(CT):
                    ps = psum_pool.tile([P, NCOLS], f32,
                                        name=f"ps{pos}_{co_t}", tag="ps")
                    for j in range(K):
                        c0, c1 = bounds[j], bounds[j + 1]
                        for ci in range(CT):
                            nc.tensor.matmul(
                                ps[:, c0:c1],
                                lhsT=wts[j][ci][:, pos, co_t, :],
                                rhs=xts[ci][:, pos, c0:c1],
                                start=(ci == 0), stop=(ci == CT - 1))
                    if pos % OCH == 0:
                        ots[co_t] = opool.tile([P, OCH, NCOLS], f16,
                                               name=f"ot{pos}_{co_t}",
                                               tag="ot")
                    ot = ots[co_t]
                    if (pos * CT + co_t) % 2 == 0:
                        nc.vector.tensor_copy(ot[:, pos % OCH, :], ps[:])
                    else:
                        nc.scalar.copy(ot[:, pos % OCH, :], ps[:])
                    if pos % OCH == OCH - 1:
                        nc.gpsimd.dma_start(out_d[co_t, pos // OCH], ot[:])
    nc.compile()
    return nc


# --------------------------- emulation -------------------------------------

def _emulate_hybrid(in_maps):
    results = []
    for im in in_maps:
        xw = im["xw"].astype(np.float32)
        ww = im["ww"].astype(np.float32)
        xd = im["xd"].astype(np.float32)
        wd = im["wd"].astype(np.float32)
        ow = np.zeros((CT, NPOS // OCH, P, OCH, WCOLS), np.float32)
        for pos in range(NPOS):
            for co_t in range(CT):
                acc = np.zeros((P, WCOLS), np.float32)
                for ci in range(CT):
                    acc += ww[ci, :, pos, co_t, :].T @ xw[ci, :, pos, :]
                ow[co_t, pos // OCH, :, pos % OCH] = acc
        od = np.zeros((ND, CT, P, H * W_), np.float32)
        for d in range(ND):
            for co_t in range(CT):
                for ch in range(2):
                    acc = np.zeros((P, 512), np.float32)
                    for ci in range(CT):
                        for tap in range(9):
                            dy, dx = divmod(tap, 3)
                            rhs = xd[d, ci][:, 16 * ch + dy:
                                            16 * ch + dy + 16,
                                            dx: dx + W_].reshape(P, 512)
                            acc += wd[d, ci, :, tap, co_t, :].T @ rhs
                    od[d, co_t, :, 512 * ch: 512 * (ch + 1)] = acc
        results.append({"ow": ow.astype(np.float16),
                        "od": od.astype(np.float16)})
    return results


def _emulate_wino(in_maps, sizes):
    bounds = [0]
    for s in sizes:
        bounds.append(bounds[-1] + s * NTILES)
    results = []
    for im in in_maps:
        xt = im["xt"].astype(np.float32)
        wt = im["wt"].astype(np.float32)
        out = np.zeros((CT, NPOS // OCH, P, OCH, 512), np.float32)
        for pos in range(NPOS):
            for co_t in range(CT):
                for j in range(len(sizes)):
                    c0, c1 = bounds[j], bounds[j + 1]
                    acc = np.zeros((P, c1 - c0), np.float32)
                    for ci in range(CT):
                        acc += wt[j, ci, :, pos, co_t, :].T @ xt[ci, :, pos,
                                                                 c0:c1]
                    out[co_t, pos // OCH, :, pos % OCH, c0:c1] = acc
        results.append({"out": out.astype(np.float16)})
    return results


# --------------------------- kernel ----------------------------------------

def _run(builder_key, build_fn, in_maps):
    global LAST_RESULTS
    from concourse.bass_utils import run_bass_kernel_spmd
    nc = _prog_cache.get(builder_key)
    if nc is None:
        nc = _prog_cache[builder_key] = build_fn()
    res = run_bass_kernel_spmd(nc, in_maps, core_ids=list(range(NCORES)),
                               trace=TRACE, tmpdir=TRACE_DIR)
    LAST_RESULTS = res
    return res.results


def _kernel_hybrid(x, arc, W, b, plan):
    wino_branch, perm = plan
    xperm = x[perm].reshape(NCORES, SPC, C, H, W_)

    U = np.einsum('xi,boaij,yj->boaxy', G, W, G)
    Uw = np.ascontiguousarray(
        U.reshape(NB, CT, P, CT, P, 6, 6)
        .transpose(0, 3, 4, 5, 6, 1, 2)
        .reshape(NB, CT, P, NPOS, CT, P)).astype(np.float16)
    # direct taps: [br, co, ci, dy, dx] -> [br, ci_t, ci, tap, co_t, co]
    Wd = np.ascontiguousarray(
        W.reshape(NB, CT, P, CT, P, 3, 3)
        .transpose(0, 3, 4, 5, 6, 1, 2)
        .reshape(NB, CT, P, 9, CT, P)).astype(np.float16)

    in_maps = []
    for c in range(NCORES):
        xw = _wino_input(xperm[c, :NW]).astype(np.float16)
        xd = np.zeros((ND, CT, P, HP, WP), np.float16)
        xd[:, :, :, 1:1 + H, 1:1 + W_] = (
            xperm[c, NW:].reshape(ND, CT, P, H, W_))
        darcs = arc[perm[SPC * c + NW: SPC * (c + 1)]]
        in_maps.append({
            "xw": np.ascontiguousarray(xw),
            "ww": np.ascontiguousarray(Uw[wino_branch[c]]),
            "xd": xd,
            "wd": np.ascontiguousarray(Wd[darcs]),
        })

    if os.environ.get("KERNEL_EMULATE") == "1":
        results = _emulate_hybrid(in_maps)
        global LAST_RESULTS
        LAST_RESULTS = None
    else:
        results = _run(("hybrid",), _build_hybrid, in_maps)

    Y = np.empty((NCORES, SPC, C, H, W_), np.float32)
    for c in range(NCORES):
        Mw = np.asarray(results[c]["ow"]).astype(np.float32)
        Mw = Mw.reshape(CT, 6, P, 6, NW, NT, NT)      # [c,xi,o,nu,n,u,v]
        yw = np.einsum('rx,cxoynuv,sy->ncourvs', AT, Mw, AT, optimize=True)
        Y[c, :NW] = yw.reshape(NW, C, H, W_)
        yd = np.asarray(results[c]["od"]).astype(np.float32)
        Y[c, NW:] = yd.reshape(ND, C, H, W_)
    Y = Y.reshape(B, C, H, W_)
    Y += b[arc[perm]][:, :, None, None]
    out = np.empty_like(Y)
    out[perm] = Y
    return out


def _kernel_wino(x, arc, W, b):
    sizes, slot_branches, perm = _plan_wino(arc)
    xperm = x[perm].reshape(NCORES, SPC, C, H, W_)

    U = np.einsum('xi,boaij,yj->boaxy', G, W, G)
    Ul = np.ascontiguousarray(
        U.reshape(NB, CT, P, CT, P, 6, 6)
        .transpose(0, 3, 4, 5, 6, 1, 2)
        .reshape(NB, CT, P, NPOS, CT, P)).astype(np.float16)
    wt = Ul[slot_branches]

    in_maps = []
    for c in range(NCORES):
        xt = _wino_input(xperm[c]).astype(np.float16)
        in_maps.append({"xt": np.ascontiguousarray(xt),
                        "wt": np.ascontiguousarray(wt[c])})

    if os.environ.get("KERNEL_EMULATE") == "1":
        results = _emulate_wino(in_maps, sizes)
        global LAST_RESULTS
        LAST_RESULTS = None
    else:
        results = _run(("wino", sizes), lambda: _build_wino(sizes), in_maps)

    M = np.stack([np.asarray(results[c]["out"]) for c in range(NCORES)])
    M32 = M.astype(np.float32).reshape(NCORES, CT, 6, P, 6, SPC, NT, NT)
    Y = np.einsum('rx,kcxoynuv,sy->kncourvs', AT, M32, AT, optimize=True)
    Y = np.ascontiguousarray(Y).reshape(B, C, H, W_)
    Y += b[arc[perm]][:, :, None, None]
    out = np.empty_like(Y)
    out[perm] = Y
    return out


def kernel(x, sample_arc, W, b):
    x = np.asarray(x, dtype=np.float32)
    arc = np.asarray(sample_arc).astype(np.int64)
    W = np.asarray(W, dtype=np.float32)
    b = np.asarray(b, dtype=np.float32)

    plan = _plan_hybrid(arc)
    if plan is not None:
        return _kernel_hybrid(x, arc, W, b, plan)
    return _kernel_wino(x, arc, W, b)


# revision 13
# speedup vs baseline: 1.0208x; 1.0208x over previous
"""MoE-routed 3x3 conv (MixedLayerWithArc) on 8 TRN2 NeuronCores.

Reference semantics: out[i] = conv3x3(x[i], W[sample_arc[i]], b[sample_arc[i]]).

Hybrid Winograd/direct kernel, DMA-roofline-driven.  The kernel is
DMA-bound (~360 GB/s/core pipe), so the split minimizes per-core HBM bytes:

  * 6 samples/core via Winograd F(4x4,3x3), all transforms on the host:
    device does 36 per-position fp16 GEMMs against ONE branch's transformed
    weights (4.72 MB).  Input inflates 2.25x but PE columns drop 4x.
  * 2 samples/core via DIRECT conv (9 shifted-window taps): raw fp16
    weights are 4x smaller (1.18 MB/sample-branch), raw x/y are 1x.
    Costs 4x PE columns for those samples, but the PE has slack.

  Per-core bytes: wino-x 7.1 + direct-x 1.2 + W (4.7+2.4) + out (8.1)
  = ~23.4 MB vs 28.3 MB for all-Winograd (5,3).  PE ~71 us, pipe ~65 us.

  Streaming structure (lessons from traces):
  * in-DMAs on SP/ACT queues in pos chunks (small first chunk -> early PE
    start); direct-sample data injected mid-stream (PE needs it last).
  * evictions alternate DVE/ACT (gpsimd can't read PSUM).
  * out-DMAs ONLY on the gpsimd (SWDGE) queue: in-order DMA queues would
    serialize them behind the in-stream and stall PSUM->PE mid-kernel.
  * On-device output transforms don't pay: DVE scalar_tensor_tensor is
    ~1.5us/op and Pool tensor ops are Q7 software (~1.2us/op).

Falls back to the all-Winograd uniform-slot program for routings where no
(6,2) packing exists.
"""
import os

import numpy as np

B, C, H, W_ = 64, 256, 32, 32
NB = 4                     # branches
NCORES = 8
SPC = B // NCORES          # samples per core
P = 128                    # partition tile
CT = C // P                # channel tiles (2)
NT = H // 4                # 8 winograd tiles per axis
NTILES = NT * NT           # 64 tiles per sample
NPOS = 36                  # 6x6 winograd positions
WARMUP = 16
OCH = 6                    # out-DMA grouping (pos per block)
IN_CHUNKS = [(0, 2), (2, 6), (6, 12), (12, 18), (18, 24), (24, 30), (30, 36)]

NW = 6                     # wino samples per core (hybrid)
ND = 2                     # direct samples per core (hybrid)
WCOLS = NW * NTILES        # 384 wino matmul columns
HP, WP = H + 2, W_ + 2     # padded spatial for the direct path

BT = np.array([
    [4, 0, -5, 0, 1, 0],
    [0, -4, -4, 1, 1, 0],
    [0, 4, -4, -1, 1, 0],
    [0, -2, -1, 2, 1, 0],
    [0, 2, -1, -2, 1, 0],
    [0, 4, 0, -5, 0, 1]], dtype=np.float32)
G = np.array([
    [1 / 4, 0, 0],
    [-1 / 6, -1 / 6, -1 / 6],
    [-1 / 6, 1 / 6, -1 / 6],
    [1 / 24, 1 / 12, 1 / 6],
    [1 / 24, -1 / 12, 1 / 6],
    [0, 0, 1]], dtype=np.float32)
AT = np.array([
    [1, 1, 1, 1, 1, 0],
    [0, 1, -1, 2, -2, 0],
    [0, 1, 1, 4, 4, 0],
    [0, 1, -1, 8, -8, 1]], dtype=np.float32)

PATTERNS = [
    (8,), (5, 3), (6, 2), (7, 1), (4, 4),
    (4, 3, 1), (3, 3, 2), (4, 2, 2), (5, 2, 1), (6, 1, 1),
    (2, 2, 2, 2), (3, 2, 2, 1), (3, 3, 1, 1), (4, 2, 1, 1), (5, 1, 1, 1),
    (2, 2, 2, 1, 1), (4, 1, 1, 1, 1), (2, 2, 1, 1, 1, 1),
    (2, 1, 1, 1, 1, 1, 1), (1,) * 8,
]

TRACE = False
TRACE_DIR = None
LAST_RESULTS = None

_prog_cache = {}


# --------------------------- routing ---------------------------------------

def _plan_hybrid(arc):
    """Per core: NW wino samples of one branch + ND direct samples (any).
    Returns (wino_branch [NCORES], perm [B]) or None if infeasible."""
    counts = np.bincount(arc, minlength=NB).tolist()

    def dfs(b, left, cs):
        if b == NB:
            return cs if left == 0 else None
        for c in range(min(left, counts[b] // NW), -1, -1):
            r = dfs(b + 1, left - c, cs + [c])
            if r is not None:
                return r
        return None

    cs = dfs(0, NCORES, [])
    if cs is None:
        return None
    wino_branch = np.array(sum(([br] * cs[br] for br in range(NB)), []),
                           np.int64)
    pools = [list(np.nonzero(arc == br)[0]) for br in range(NB)]
    perm = np.empty(B, np.int64)
    for c in range(NCORES):
        br = wino_branch[c]
        for k in range(NW):
            perm[SPC * c + k] = pools[br].pop()
    rest = [i for br in range(NB) for i in pools[br]]
    for c in range(NCORES):
        for k in range(ND):
            perm[SPC * c + NW + k] = rest[ND * c + k]
    return wino_branch, perm


def _pattern_assign(counts, sizes):
    K = len(sizes)
    caps = [NCORES] * K
    m = []

    def branch_vecs(j, rem):
        if j == K:
            if rem == 0:
                yield []
            return
        for v in range(min(caps[j], rem // sizes[j]), -1, -1):
            for rest in branch_vecs(j + 1, rem - v * sizes[j]):
                yield [v] + rest

    def dfs(b):
        if b == NB:
            return all(c == 0 for c in caps)
        for v in branch_vecs(0, counts[b]):
            for j in range(K):
                caps[j] -= v[j]
            m.append(v)
            if dfs(b + 1):
                return True
            m.pop()
            for j in range(K):
                caps[j] += v[j]
        return False

    return m if dfs(0) else None


def _plan_wino(arc):
    counts = np.bincount(arc, minlength=NB).tolist()
    for sizes in PATTERNS:
        m = _pattern_assign(counts, sizes)
        if m is not None:
            break
    assert m is not None
    K = len(sizes)
    slot_branches = np.empty((NCORES, K), np.int64)
    for j in range(K):
        lst = []
        for br in range(NB):
            lst += [br] * m[br][j]
        slot_branches[:, j] = lst
    pools = [list(np.nonzero(arc == br)[0]) for br in range(NB)]
    perm = np.empty(B, np.int64)
    i = 0
    for c in range(NCORES):
        for j, sz in enumerate(sizes):
            br = slot_branches[c, j]
            for _ in range(sz):
                perm[i] = pools[br].pop()
                i += 1
    return sizes, slot_branches, perm


# --------------------------- host transforms -------------------------------

def _wino_input(x_sel):
    """[n,C,H,W] -> transformed V [n,CT,P,36, n?..]: returns
    [CT, P, 36, n*64] f16 stacked per core by the caller."""
    n = x_sel.shape[0]
    xp = np.zeros((n, C, H + 2, W_ + 2), np.float32)
    xp[:, :, 1:1 + H, 1:1 + W_] = x_sel
    win = np.lib.stride_tricks.sliding_window_view(
        xp, (6, 6), axis=(2, 3))[:, :, ::4, ::4]      # [n,C,8,8,6,6]
    V = np.matmul(np.matmul(BT, win), BT.T)           # [n,C,8,8,xi,nu]
    V = V.reshape(n, CT, P, NT, NT, 6, 6)
    # -> [CT, P, pos, n, ty, tx]
    return V.transpose(1, 2, 5, 6, 0, 3, 4).reshape(CT, P, NPOS, n * NTILES)


# --------------------------- device programs -------------------------------

def _build_hybrid():
    import concourse.tile as tile
    from concourse import bacc, mybir

    nc = bacc.Bacc("TRN2", target_bir_lowering=False, debug=False,
                   num_devices=NCORES)
    f32 = mybir.dt.float32
    f16 = mybir.dt.float16

    xw_d = nc.dram_tensor("xw", [CT, P, NPOS, WCOLS], f16,
                          kind="ExternalInput").ap()
    ww_d = nc.dram_tensor("ww", [CT, P, NPOS, CT, P], f16,
                          kind="ExternalInput").ap()
    xd_d = nc.dram_tensor("xd", [ND, CT, P, HP, WP], f16,
                          kind="ExternalInput").ap()
    wd_d = nc.dram_tensor("wd", [ND, CT, P, 9, CT, P], f16,
                          kind="ExternalInput").ap()
    ow_d = nc.dram_tensor("ow", [CT, NPOS // OCH, P, OCH, WCOLS], f16,
                          kind="ExternalOutput").ap()
    od_d = nc.dram_tensor("od", [ND, CT, P, H * W_], f16,
                          kind="ExternalOutput").ap()

    with tile.TileContext(nc) as tc:
        with tc.tile_pool(name="xpool", bufs=1) as xpool, \
             tc.tile_pool(name="wpool", bufs=1) as wpool, \
             tc.tile_pool(name="opool", bufs=4) as opool, \
             tc.tile_pool(name="psum", bufs=8, space="PSUM") as psum_pool:

            scratch = xpool.tile([P, P], f16, name="scratch", tag="scr")
            nc.gpsimd.memset(scratch[:], 0.0)
            ps_warm = psum_pool.tile([P, 512], f32, name="ps_warm", tag="ps")
            for _ in range(WARMUP):
                nc.tensor.matmul(ps_warm[:, :P], scratch[:], scratch[:],
                                 start=True, stop=True, skip_group_check=True)

            xws = [xpool.tile([P, NPOS, WCOLS], f16, name=f"xw{ci}",
                              tag=f"xw{ci}") for ci in range(CT)]
            wws = [wpool.tile([P, NPOS, CT, P], f16, name=f"ww{ci}",
                              tag=f"ww{ci}") for ci in range(CT)]
            xds = [[xpool.tile([P, HP, WP], f16, name=f"xd{d}_{ci}",
                               tag=f"xd{d}_{ci}") for ci in range(CT)]
                   for d in range(ND)]
            wds = [[wpool.tile([P, 9, CT, P], f16, name=f"wd{d}_{ci}",
                               tag=f"wd{d}_{ci}") for ci in range(CT)]
                   for d in range(ND)]

            for k, (lo, hi) in enumerate(IN_CHUNKS):
                sl = slice(lo, hi)
                for ci in range(CT):
                    nc.sync.dma_start(xws[ci][:, sl], xw_d[ci][:, sl])
                    nc.scalar.dma_start(wws[ci][:, sl], ww_d[ci][:, sl])
                if k == 1:
                    # direct-path data lands early: its banks interleave as
                    # PE filler while the wino in-stream (which alone
                    # demands ~485 GB/s of input vs the ~360 GB/s pipe)
                    # catches up
                    for d in range(ND):
                        for ci in range(CT):
                            nc.sync.dma_start(xds[d][ci][:], xd_d[d, ci])
                            nc.scalar.dma_start(wds[d][ci][:], wd_d[d, ci])

            def direct_pair(u):
                # one (d, co_t) unit: 2 PSUM banks of 18 accumulating
                # matmuls + grouped y out-DMA
                d, co_t = divmod(u, CT)
                oy = opool.tile([P, H * W_], f16, name=f"oy{d}_{co_t}",
                                tag="oy")
                for ch in range(2):
                    ps = psum_pool.tile([P, 512], f32,
                                        name=f"pd{d}_{co_t}_{ch}", tag="ps")
                    for ci in range(CT):
                        for tap in range(9):
                            dy, dx = divmod(tap, 3)
                            rhs = xds[d][ci][:, 16 * ch + dy:
                                             16 * ch + dy + 16, dx: dx + W_]
                            nc.tensor.matmul(
                                ps[:],
                                lhsT=wds[d][ci][:, tap, co_t, :],
                                rhs=rhs,
                                start=(ci == 0 and tap == 0),
                                stop=(ci == CT - 1 and tap == 8))
                    if (d * CT + co_t + ch) % 2 == 0:
                        nc.vector.tensor_copy(
                            oy[:, 512 * ch: 512 * (ch + 1)], ps[:])
                    else:
                        nc.scalar.copy(
                            oy[:, 512 * ch: 512 * (ch + 1)], ps[:])
                nc.gpsimd.dma_start(od_d[d, co_t], oy[:])

            # wino 6-pos groups with direct pair-units interleaved at the
            # group boundaries (one after each of groups 1..4)
            ots = {}
            unit = 0
            for pos in range(NPOS):
                for co_t in range(CT):
                    ps = psum_pool.tile([P, 512], f32,
                                        name=f"ps{pos}_{co_t}", tag="ps")
                    for ci in range(CT):
                        nc.tensor.matmul(
                            ps[:, :WCOLS],
                            lhsT=wws[ci][:, pos, co_t, :],
                            rhs=xws[ci][:, pos, :],
                            start=(ci == 0), stop=(ci == CT - 1))
                    if pos % OCH == 0:
                        ots[co_t] = opool.tile([P, OCH, WCOLS], f16,
                                               name=f"ot{pos}_{co_t}",
                                               tag="ot")
                    ot = ots[co_t]
                    if (pos * CT + co_t) % 2 == 0:
                        nc.vector.tensor_copy(ot[:, pos % OCH, :],
                                              ps[:, :WCOLS])
                    else:
                        nc.scalar.copy(ot[:, pos % OCH, :], ps[:, :WCOLS])
                    if pos % OCH == OCH - 1:
                        nc.gpsimd.dma_start(ow_d[co_t, pos // OCH], ot[:])
                if pos % OCH == OCH - 1 and 1 <= pos // OCH <= 4:
                    direct_pair(unit)
                    unit += 1
            while unit < ND * CT:
                direct_pair(unit)
                unit += 1
    nc.compile()
    return nc


def _build_wino(sizes):
    import concourse.tile as tile
    from concourse import bacc, mybir

    K = len(sizes)
    NCOLS = 512
    bounds = [0]
    for s in sizes:
        bounds.append(bounds[-1] + s * NTILES)

    nc = bacc.Bacc("TRN2", target_bir_lowering=False, debug=False,
                   num_devices=NCORES)
    f32 = mybir.dt.float32
    f16 = mybir.dt.float16

    xt_d = nc.dram_tensor("xt", [CT, P, NPOS, NCOLS], f16,
                          kind="ExternalInput").ap()
    wt_d = nc.dram_tensor("wt", [K, CT, P, NPOS, CT, P], f16,
                          kind="ExternalInput").ap()
    out_d = nc.dram_tensor("out", [CT, NPOS // OCH, P, OCH, NCOLS], f16,
                           kind="ExternalOutput").ap()

    with tile.TileContext(nc) as tc:
        with tc.tile_pool(name="xpool", bufs=1) as xpool, \
             tc.tile_pool(name="wpool", bufs=1) as wpool, \
             tc.tile_pool(name="opool", bufs=4) as opool, \
             tc.tile_pool(name="psum", bufs=8, space="PSUM") as psum_pool:

            scratch = xpool.tile([P, P], f16, name="scratch", tag="scr")
            nc.gpsimd.memset(scratch[:], 0.0)
            ps_warm = psum_pool.tile([P, NCOLS], f32, name="ps_warm", tag="ps")
            for _ in range(WARMUP):
                nc.tensor.matmul(ps_warm[:, :P], scratch[:], scratch[:],
                                 start=True, stop=True, skip_group_check=True)

            xts = [xpool.tile([P, NPOS, NCOLS], f16, name=f"xt{ci}",
                              tag=f"xt{ci}") for ci in range(CT)]
            wts = [[wpool.tile([P, NPOS, CT, P], f16, name=f"wt{j}_{ci}",
                               tag=f"wt{j}_{ci}")
                    for ci in range(CT)] for j in range(K)]

            for lo, hi in IN_CHUNKS:
                sl = slice(lo, hi)
                for ci in range(CT):
                    nc.sync.dma_start(xts[ci][:, sl], xt_d[ci][:, sl])
                for j in range(K):
                    for ci in range(CT):
                        nc.scalar.dma_start(wts[j][ci][:, sl],
                                            wt_d[j, ci][:, sl])

            ots = {}
            for pos in range(NPOS):
                for co_t in range(CT):
                    ps = psum_pool.tile([P, NCOLS], f32,
                                        name=f"ps{pos}_{co_t}", tag="ps")
                    for j in range(K):
                        c0, c1 = bounds[j], bounds[j + 1]
                        for ci in range(CT):
                            nc.tensor.matmul(
                                ps[:, c0:c1],
                                lhsT=wts[j][ci][:, pos, co_t, :],
                                rhs=xts[ci][:, pos, c0:c1],
                                start=(ci == 0), stop=(ci == CT - 1))
                    if pos % OCH == 0:
                        ots[co_t] = opool.tile([P, OCH, NCOLS], f16,
                                               name=f"ot{pos}_{co_t}",
                                               tag="ot")
                    ot = ots[co_t]
                    if (pos * CT + co_t) % 2 == 0:
                        nc.vector.tensor_copy(ot[:, pos % OCH, :], ps[:])
                    else:
                        nc.scalar.copy(ot[:, pos % OCH, :], ps[:])
                    if pos % OCH == OCH - 1:
                        nc.gpsimd.dma_start(out_d[co_t, pos // OCH], ot[:])
    nc.compile()
    return nc


# --------------------------- emulation -------------------------------------

def _emulate_hybrid(in_maps):
    results = []
    for im in in_maps:
        xw = im["xw"].astype(np.float32)
        ww = im["ww"].astype(np.float32)
        xd = im["xd"].astype(np.float32)
        wd = im["wd"].astype(np.float32)
        ow = np.zeros((CT, NPOS // OCH, P, OCH, WCOLS), np.float32)
        for pos in range(NPOS):
            for co_t in range(CT):
                acc = np.zeros((P, WCOLS), np.float32)
                for ci in range(CT):
                    acc += ww[ci, :, pos, co_t, :].T @ xw[ci, :, pos, :]
                ow[co_t, pos // OCH, :, pos % OCH] = acc
        od = np.zeros((ND, CT, P, H * W_), np.float32)
        for d in range(ND):
            for co_t in range(CT):
                for ch in range(2):
                    acc = np.zeros((P, 512), np.float32)
                    for ci in range(CT):
                        for tap in range(9):
                            dy, dx = divmod(tap, 3)
                            rhs = xd[d, ci][:, 16 * ch + dy:
                                            16 * ch + dy + 16,
                                            dx: dx + W_].reshape(P, 512)
                            acc += wd[d, ci, :, tap, co_t, :].T @ rhs
                    od[d, co_t, :, 512 * ch: 512 * (ch + 1)] = acc
        results.append({"ow": ow.astype(np.float16),
                        "od": od.astype(np.float16)})
    return results


def _emulate_wino(in_maps, sizes):
    bounds = [0]
    for s in sizes:
        bounds.append(bounds[-1] + s * NTILES)
    results = []
    for im in in_maps:
        xt = im["xt"].astype(np.float32)
        wt = im["wt"].astype(np.float32)
        out = np.zeros((CT, NPOS // OCH, P, OCH, 512), np.float32)
        for pos in range(NPOS):
            for co_t in range(CT):
                for j in range(len(sizes)):
                    c0, c1 = bounds[j], bounds[j + 1]
                    acc = np.zeros((P, c1 - c0), np.float32)
                    for ci in range(CT):
                        acc += wt[j, ci, :, pos, co_t, :].T @ xt[ci, :, pos,
                                                                 c0:c1]
                    out[co_t, pos // OCH, :, pos % OCH, c0:c1] = acc
        results.append({"out": out.astype(np.float16)})
    return results


# --------------------------- kernel ----------------------------------------

def _run(builder_key, build_fn, in_maps):
    global LAST_RESULTS
    from concourse.bass_utils import run_bass_kernel_spmd
    nc = _prog_cache.get(builder_key)
    if nc is None:
        nc = _prog_cache[builder_key] = build_fn()
    res = run_bass_kernel_spmd(nc, in_maps, core_ids=list(range(NCORES)),
                               trace=TRACE, tmpdir=TRACE_DIR)
    LAST_RESULTS = res
    return res.results


def _kernel_hybrid(x, arc, W, b, plan):
    wino_branch, perm = plan
    xperm = x[perm].reshape(NCORES, SPC, C, H, W_)

    U = np.einsum('xi,boaij,yj->boaxy', G, W, G)
    Uw = np.ascontiguousarray(
        U.reshape(NB, CT, P, CT, P, 6, 6)
        .transpose(0, 3, 4, 5, 6, 1, 2)
        .reshape(NB, CT, P, NPOS, CT, P)).astype(np.float16)
    # direct taps: [br, co, ci, dy, dx] -> [br, ci_t, ci, tap, co_t, co]
    Wd = np.ascontiguousarray(
        W.reshape(NB, CT, P, CT, P, 3, 3)
        .transpose(0, 3, 4, 5, 6, 1, 2)
        .reshape(NB, CT, P, 9, CT, P)).astype(np.float16)

    in_maps = []
    for c in range(NCORES):
        xw = _wino_input(xperm[c, :NW]).astype(np.float16)
        xd = np.zeros((ND, CT, P, HP, WP), np.float16)
        xd[:, :, :, 1:1 + H, 1:1 + W_] = (
            xperm[c, NW:].reshape(ND, CT, P, H, W_))
        darcs = arc[perm[SPC * c + NW: SPC * (c + 1)]]
        in_maps.append({
            "xw": np.ascontiguousarray(xw),
            "ww": np.ascontiguousarray(Uw[wino_branch[c]]),
            "xd": xd,
            "wd": np.ascontiguousarray(Wd[darcs]),
        })

    if os.environ.get("KERNEL_EMULATE") == "1":
        results = _emulate_hybrid(in_maps)
        global LAST_RESULTS
        LAST_RESULTS = None
    else:
        results = _run(("hybrid",), _build_hybrid, in_maps)

    Y = np.empty((NCORES, SPC, C, H, W_), np.float32)
    for c in range(NCORES):
        Mw = np.asarray(results[c]["ow"]).astype(np.float32)
        Mw = Mw.reshape(CT, 6, P, 6, NW, NT, NT)      # [c,xi,o,nu,n,u,v]
        yw = np.einsum('rx,cxoynuv,sy->ncourvs', AT, Mw, AT, optimize=True)
        Y[c, :NW] = yw.reshape(NW, C, H, W_)
        yd = np.asarray(results[c]["od"]).astype(np.float32)
        Y[c, NW:] = yd.reshape(ND, C, H, W_)
    Y = Y.reshape(B, C, H, W_)
    Y += b[arc[perm]][:, :, None, None]
    out = np.empty_like(Y)
    out[perm] = Y
    return out


def _kernel_wino(x, arc, W, b):
    sizes, slot_branches, perm = _plan_wino(arc)
    xperm = x[perm].reshape(NCORES, SPC, C, H, W_)

    U = np.einsum('xi,boaij,yj->boaxy', G, W, G)
    Ul = np.ascontiguousarray(
        U.reshape(NB, CT, P, CT, P, 6, 6)
        .transpose(0, 3, 4, 5, 6, 1, 2)
        .reshape(NB, CT, P, NPOS, CT, P)).astype(np.float16)
    wt = Ul[slot_branches]

    in_maps = []
    for c in range(NCORES):
        xt = _wino_input(xperm[c]).astype(np.float16)
        in_maps.append({"xt": np.ascontiguousarray(xt),
                        "wt": np.ascontiguousarray(wt[c])})

    if os.environ.get("KERNEL_EMULATE") == "1":
        results = _emulate_wino(in_maps, sizes)
        global LAST_RESULTS
        LAST_RESULTS = None
    else:
        results = _run(("wino", sizes), lambda: _build_wino(sizes), in_maps)

    M = np.stack([np.asarray(results[c]["out"]) for c in range(NCORES)])
    M32 = M.astype(np.float32).reshape(NCORES, CT, 6, P, 6, SPC, NT, NT)
    Y = np.einsum('rx,kcxoynuv,sy->kncourvs', AT, M32, AT, optimize=True)
    Y = np.ascontiguousarray(Y).reshape(B, C, H, W_)
    Y += b[arc[perm]][:, :, None, None]
    out = np.empty_like(Y)
    out[perm] = Y
    return out


def kernel(x, sample_arc, W, b):
    x = np.asarray(x, dtype=np.float32)
    arc = np.asarray(sample_arc).astype(np.int64)
    W = np.asarray(W, dtype=np.float32)
    b = np.asarray(b, dtype=np.float32)

    plan = _plan_hybrid(arc)
    if plan is not None:
        return _kernel_hybrid(x, arc, W, b, plan)
    return _kernel_wino(x, arc, W, b)


# revision 14
# speedup vs baseline: 1.1823x; 1.1582x over previous
"""MoE-routed 3x3 conv (MixedLayerWithArc) on 8 TRN2 NeuronCores.

Reference semantics: out[i] = conv3x3(x[i], W[sample_arc[i]], b[sample_arc[i]]).

Strategy (Winograd F(4x4,3x3), all transforms on the host):
  * Routing resolved on the HOST: each sample runs 1 conv with its selected
    branch weights.  Samples are packed so every core's 8 samples group into
    K uniform-size single-branch "slots" (sizes searched from the observed
    arc; (5,3) for the test routing) -> one stationary weight per
    (slot, pos, ci_t, co_t) matmul group.
  * Winograd F(4,3): y = A^T [ (G w G^T) o (B^T d B) ] A.  Input, weight AND
    output transforms run on the host in numpy; the device does only the
    36 per-position fp16 GEMMs (1 col/cycle, 2.25x fewer PE columns than
    direct conv) plus PSUM->SBUF fp16 eviction and DMAs.
  * The kernel is DMA-bound (28.3 MB/core over a ~360 GB/s/core pipe):
      - in-DMAs (xt 9.4 MB + wt 9.4 MB) stream on the SP/ACT queues in pos
        chunks (small first chunk so the PE starts early);
      - evictions alternate DVE/ACT into per-6-pos grouped f16 tiles;
      - out-DMAs (9.4 MB) go ONLY on the gpsimd (SWDGE) queue so they
        interleave with the in-stream instead of queueing behind it
        (the phase-1 lesson: in-order DMA queues serialize out after in,
        stalling PSUM eviction and the PE mid-kernel).
  * On-device output transforms don't pay: DVE scalar_tensor_tensor is
    ~1.5us/op and Pool tensor ops are Q7 software (~1.2us/op), so the
    xi-pass costs ~100us of engine time to save 9.4us of DMA.

Per-core tensors:
  xt  [2, 128, 36, 512] f16      (ci_t, ci, pos=xi*6+nu, samp*64+tile)
  wt  [K, 2, 128, 36, 2, 128] f16 (slot, ci_t, ci, pos, co_t, co)
  out [2, 6, 128, 6, 512] f16    (co_t, chunk, co, pos%6, samp*64+tile)
"""
import os

import numpy as np

B, C, H, W_ = 64, 256, 32, 32
NB = 4                     # branches
NCORES = 8
SPC = B // NCORES          # samples per core
P = 128                    # partition tile
CT = C // P                # channel tiles (2)
NT = H // 4                # 8 winograd tiles per axis
NTILES = NT * NT           # 64 tiles per sample
NPOS = 36                  # 6x6 winograd positions
NCOLS = SPC * NTILES       # 512 = one PSUM bank of fp32
WARMUP = 16
OCH = 6                    # out-DMA grouping (pos per block)
# in-DMA chunking over the pos axis; small first chunk -> early PE start
IN_CHUNKS = [(0, 2), (2, 6), (6, 12), (12, 18), (18, 24), (24, 30), (30, 36)]

BT = np.array([
    [4, 0, -5, 0, 1, 0],
    [0, -4, -4, 1, 1, 0],
    [0, 4, -4, -1, 1, 0],
    [0, -2, -1, 2, 1, 0],
    [0, 2, -1, -2, 1, 0],
    [0, 4, 0, -5, 0, 1]], dtype=np.float32)
G = np.array([
    [1 / 4, 0, 0],
    [-1 / 6, -1 / 6, -1 / 6],
    [-1 / 6, 1 / 6, -1 / 6],
    [1 / 24, 1 / 12, 1 / 6],
    [1 / 24, -1 / 12, 1 / 6],
    [0, 0, 1]], dtype=np.float32)
AT = np.array([
    [1, 1, 1, 1, 1, 0],
    [0, 1, -1, 2, -2, 0],
    [0, 1, 1, 4, 4, 0],
    [0, 1, -1, 8, -8, 1]], dtype=np.float32)

# slot-size patterns in cost order (fewer slots = less weight DMA); the
# trailing patterns guarantee feasibility for any routing.
PATTERNS = [
    (8,), (5, 3), (6, 2), (7, 1), (4, 4),
    (4, 3, 1), (3, 3, 2), (4, 2, 2), (5, 2, 1), (6, 1, 1),
    (2, 2, 2, 2), (3, 2, 2, 1), (3, 3, 1, 1), (4, 2, 1, 1), (5, 1, 1, 1),
    (2, 2, 2, 1, 1), (4, 1, 1, 1, 1), (2, 2, 1, 1, 1, 1),
    (2, 1, 1, 1, 1, 1, 1), (1,) * 8,
]

TRACE = False
TRACE_DIR = None
LAST_RESULTS = None

_prog_cache = {}


def _pattern_assign(counts, sizes):
    """m[b][j] = #cores whose slot j holds branch b, such that every slot is
    filled on all 8 cores and every branch's samples are exactly consumed."""
    K = len(sizes)
    caps = [NCORES] * K
    m = []

    def branch_vecs(j, rem):
        if j == K:
            if rem == 0:
                yield []
            return
        for v in range(min(caps[j], rem // sizes[j]), -1, -1):
            for rest in branch_vecs(j + 1, rem - v * sizes[j]):
                yield [v] + rest

    def dfs(b):
        if b == NB:
            return all(c == 0 for c in caps)
        for v in branch_vecs(0, counts[b]):
            for j in range(K):
                caps[j] -= v[j]
            m.append(v)
            if dfs(b + 1):
                return True
            m.pop()
            for j in range(K):
                caps[j] += v[j]
        return False

    return m if dfs(0) else None


def _plan_routing(arc):
    counts = np.bincount(arc, minlength=NB).tolist()
    for sizes in PATTERNS:
        m = _pattern_assign(counts, sizes)
        if m is not None:
            break
    assert m is not None
    K = len(sizes)
    slot_branches = np.empty((NCORES, K), np.int64)
    for j in range(K):
        lst = []
        for br in range(NB):
            lst += [br] * m[br][j]
        slot_branches[:, j] = lst
    pools = [list(np.nonzero(arc == br)[0]) for br in range(NB)]
    perm = np.empty(B, np.int64)
    i = 0
    for c in range(NCORES):
        for j, sz in enumerate(sizes):
            br = slot_branches[c, j]
            for _ in range(sz):
                perm[i] = pools[br].pop()
                i += 1
    return sizes, slot_branches, perm


def _build_program(sizes):
    import concourse.tile as tile
    from concourse import bacc, mybir

    K = len(sizes)
    bounds = [0]
    for s in sizes:
        bounds.append(bounds[-1] + s * NTILES)

    nc = bacc.Bacc("TRN2", target_bir_lowering=False, debug=False,
                   num_devices=NCORES)
    f32 = mybir.dt.float32
    f16 = mybir.dt.float16

    xt_d = nc.dram_tensor("xt", [CT, P, NPOS, NCOLS], f16,
                          kind="ExternalInput").ap()
    wt_d = nc.dram_tensor("wt", [K, CT, P, NPOS, CT, P], f16,
                          kind="ExternalInput").ap()
    out_d = nc.dram_tensor("out", [CT, NPOS // OCH, P, OCH, NCOLS], f16,
                           kind="ExternalOutput").ap()

    with tile.TileContext(nc) as tc:
        with tc.tile_pool(name="xpool", bufs=1) as xpool, \
             tc.tile_pool(name="wpool", bufs=1) as wpool, \
             tc.tile_pool(name="opool", bufs=4) as opool, \
             tc.tile_pool(name="psum", bufs=8, space="PSUM") as psum_pool:

            # PE warmup: dummy matmuls during the initial DMA fill so the
            # p-state clock ramps before the first real matmul.
            scratch = xpool.tile([P, P], f16, name="scratch", tag="scr")
            nc.gpsimd.memset(scratch[:], 0.0)
            ps_warm = psum_pool.tile([P, NCOLS], f32, name="ps_warm", tag="ps")
            for _ in range(WARMUP):
                nc.tensor.matmul(ps_warm[:, :P], scratch[:], scratch[:],
                                 start=True, stop=True, skip_group_check=True)

            xts = [xpool.tile([P, NPOS, NCOLS], f16, name=f"xt{ci}",
                              tag=f"xt{ci}") for ci in range(CT)]
            wts = [[wpool.tile([P, NPOS, CT, P], f16, name=f"wt{j}_{ci}",
                               tag=f"wt{j}_{ci}")
                    for ci in range(CT)] for j in range(K)]

            for lo, hi in IN_CHUNKS:
                sl = slice(lo, hi)
                for ci in range(CT):
                    nc.sync.dma_start(xts[ci][:, sl], xt_d[ci][:, sl])
                for j in range(K):
                    for ci in range(CT):
                        nc.scalar.dma_start(wts[j][ci][:, sl],
                                            wt_d[j, ci][:, sl])

            ots = {}
            for pos in range(NPOS):
                for co_t in range(CT):
                    ps = psum_pool.tile([P, NCOLS], f32,
                                        name=f"ps{pos}_{co_t}", tag="ps")
                    for j in range(K):
                        c0, c1 = bounds[j], bounds[j + 1]
                        for ci in range(CT):
                            nc.tensor.matmul(
                                ps[:, c0:c1],
                                lhsT=wts[j][ci][:, pos, co_t, :],
                                rhs=xts[ci][:, pos, c0:c1],
                                start=(ci == 0), stop=(ci == CT - 1))
                    if pos % OCH == 0:
                        ots[co_t] = opool.tile([P, OCH, NCOLS], f16,
                                               name=f"ot{pos}_{co_t}",
                                               tag="ot")
                    ot = ots[co_t]
                    # evictions alternate DVE/ACT (gpsimd can't read PSUM)
                    if (pos * CT + co_t) % 2 == 0:
                        nc.vector.tensor_copy(ot[:, pos % OCH, :], ps[:])
                    else:
                        nc.scalar.copy(ot[:, pos % OCH, :], ps[:])
                    if pos % OCH == OCH - 1:
                        # grouped 6-pos block on the SWDGE queue: never
                        # queues behind the in-stream
                        nc.gpsimd.dma_start(out_d[co_t, pos // OCH], ot[:])
    nc.compile()
    return nc


def _emulate(in_maps, sizes):
    """Numpy stand-in for the device program (layout/packing validation)."""
    bounds = [0]
    for s in sizes:
        bounds.append(bounds[-1] + s * NTILES)
    results = []
    for im in in_maps:
        xt = im["xt"].astype(np.float32)
        wt = im["wt"].astype(np.float32)
        out = np.zeros((CT, NPOS // OCH, P, OCH, NCOLS), np.float32)
        for pos in range(NPOS):
            for co_t in range(CT):
                for j in range(len(sizes)):
                    c0, c1 = bounds[j], bounds[j + 1]
                    acc = np.zeros((P, c1 - c0), np.float32)
                    for ci in range(CT):
                        acc += wt[j, ci, :, pos, co_t, :].T @ xt[ci, :, pos,
                                                                 c0:c1]
                    out[co_t, pos // OCH, :, pos % OCH, c0:c1] = acc
        results.append({"out": out.astype(np.float16)})
    return results


def kernel(x, sample_arc, W, b):
    global LAST_RESULTS

    x = np.asarray(x, dtype=np.float32)
    arc = np.asarray(sample_arc).astype(np.int64)
    W = np.asarray(W, dtype=np.float32)
    b = np.asarray(b, dtype=np.float32)

    sizes, slot_branches, perm = _plan_routing(arc)

    # ---- host input transform: V = B^T d B over 6x6 windows, stride 4 ----
    xp = np.zeros((B, C, H + 2, W_ + 2), np.float32)
    xp[:, :, 1:1 + H, 1:1 + W_] = x
    win = np.lib.stride_tricks.sliding_window_view(
        xp, (6, 6), axis=(2, 3))[:, :, ::4, ::4]      # [B,C,8,8,6,6]
    V = np.matmul(np.matmul(BT, win), BT.T)           # [B,C,8,8,xi,nu]
    Vp = V[perm].reshape(NCORES, SPC, CT, P, NT, NT, 6, 6)
    xt = np.ascontiguousarray(
        Vp.transpose(0, 2, 3, 6, 7, 1, 4, 5).reshape(
            NCORES, CT, P, NPOS, NCOLS)).astype(np.float16)

    # ---- host weight transform: U = G w G^T ----
    U = np.einsum('xi,boaij,yj->boaxy', G, W, G)      # [NB,co,ci,xi,nu]
    Ul = np.ascontiguousarray(
        U.reshape(NB, CT, P, CT, P, 6, 6)
        .transpose(0, 3, 4, 5, 6, 1, 2)
        .reshape(NB, CT, P, NPOS, CT, P)).astype(np.float16)
    wt = Ul[slot_branches]                            # [NCORES,K,CT,P,36,CT,P]

    in_maps = [{"xt": xt[c], "wt": np.ascontiguousarray(wt[c])}
               for c in range(NCORES)]

    if os.environ.get("KERNEL_EMULATE") == "1":
        results = _emulate(in_maps, sizes)
        LAST_RESULTS = None
    else:
        from concourse.bass_utils import run_bass_kernel_spmd
        key = sizes
        nc = _prog_cache.get(key)
        if nc is None:
            nc = _prog_cache[key] = _build_program(sizes)
        res = run_bass_kernel_spmd(nc, in_maps, core_ids=list(range(NCORES)),
                                   trace=TRACE, tmpdir=TRACE_DIR)
        LAST_RESULTS = res
        results = res.results

    # ---- host output transform: Y = A^T M A, + bias, un-permute ----
    M = np.stack([np.asarray(results[c]["out"]) for c in range(NCORES)])
    # [core, co_t, chunk, co, pos%6, cols] -> pos = chunk*6 + i = xi*6 + nu
    M32 = M.astype(np.float32).reshape(NCORES, CT, 6, P, 6, SPC, NT, NT)
    Y = np.einsum('rx,kcxoynuv,sy->kncourvs', AT, M32, AT, optimize=True)
    Y = np.ascontiguousarray(Y).reshape(B, C, H, W_)
    Y += b[arc[perm]][:, :, None, None]
    out = np.empty_like(Y)
    out[perm] = Y
    return out
